# revision 5
# baseline (speedup 1.0000x reference)
"""Trainium2 Bass kernel for nn_CDEModel (neural CDE, RK4 over cubic-spline control).

Strategy (pure data parallel over batch, 8 cores x 512 rows):
  * Host precomputes G_u [127,128] matrices mapping knot values x -> spline
    derivative dX(u) per interval for the 5 RK4 sample points u.
  * Phase 1: transpose-load x = [t, a] to T-layout xT [128L, ch*512b] (fp32).
  * Phase 2: dX_u = G_u matmuls (fp32) -> compact DRAM scratch dxall
    [40=(ch-1)*5+u, NI, BC] bf16 (action channels only; the t channel has
    dX == 1 exactly and is folded via the W10/W2x0 matmuls directly).
  * Phase 3: encoder z0 -> zT [64, BC].
  * Scan (127 intervals x 2 substeps x 4 RK stages). Per interval the
    compact dX slice dxc [40, BC] (40 KB) is DMA-prefetched two intervals
    ahead; the 64-way latent broadcast needed by the pair multiplies is
    done ON-CORE by the PE: bt(pt,u) [128, BC] = ST(pt,u)^T @ dxc where
    ST is a 0/1 selection stationary. This removes the per-interval 2.5 MB
    replicating DMA of the previous design (the HW bottleneck).
    RK bookkeeping is algebraically folded into PE accumulations:
      hpre(s+1) = W1@z + alpha_s*W1@k_s accumulates directly from the pair
      products psb via W1P = Spair@W1 and from hsb via W10 = W2x0@W1;
      znew accumulates in its own PSUM group via beta_s-scaled Spair/W2x0.
      Per stage: pair-chunk mm2 (PE) -> f*dX (GPSIMD x2 / DVE x2)
      -> accumulating matmuls. Broadcast matmuls for interval i+1 are
      interleaved into interval i's stages to fill PE gaps.
  * Decoder matmul per interval; output staged T-layout via the ACT DGE
    queue; host transposes.
"""

import sys

import numpy as np

sys.path.insert(0, "/opt/trn_rl_repo")

# ---- problem constants (hardcoded per contract) ----
B = 4096
L = 128
SD = 32          # state dim
AD = 8           # action dim
LD = 64          # latent dim
HID = 128        # hidden dim
XD = AD + 1      # control channels (t + actions)
NCORES = 8
BC = B // NCORES          # 512 batch rows per core
NI = L - 1                # 127 intervals
NU = 5                    # u grid {0,.25,.5,.75,1}
UVALS = [0.0, 0.25, 0.5, 0.75, 1.0]

_CACHE = {}
LAST_RESULTS = None


def _host_consts():
    n = L - 2
    M = 4.0 * np.eye(n) + np.eye(n, k=1) + np.eye(n, k=-1)
    Minv = np.linalg.inv(M)
    D2 = np.zeros((n, L))
    for i in range(n):
        D2[i, i], D2[i, i + 1], D2[i, i + 2] = 1.0, -2.0, 1.0
    Pfull = np.zeros((L, L))
    Pfull[1:L - 1, :] = 6.0 * (Minv @ D2)
    S0 = np.eye(L)[: L - 1, :]
    S1 = np.eye(L)[1:, :]
    Delta = np.zeros((L - 1, L))
    for i in range(L - 1):
        Delta[i, i], Delta[i, i + 1] = -1.0, 1.0
    Gt = np.zeros((NU, L, NI), np.float32)
    for ui, u in enumerate(UVALS):
        al = -1.0 / 3.0 + u - u * u / 2.0
        be = -1.0 / 6.0 + u * u / 2.0
        G = Delta + al * (S0 @ Pfull) + be * (S1 @ Pfull)
        Gt[ui] = G.T.astype(np.float32)

    ident = np.eye(L, dtype=np.float32)
    I64 = np.eye(64, dtype=np.float32)
    Spair = np.concatenate([I64, I64], axis=0)   # [128, 64]

    # broadcast-selection stationaries: ST[(pt*NU+u)] maps dxc [40, BC] ->
    # bt [128, BC] with partitions 0:64 = dX[ch=1+2pt], 64:128 = dX[ch=2+2pt]
    STbc = np.zeros((40, 4 * NU * 128), np.float32)
    for pt in range(4):
        for u in range(NU):
            idx = pt * NU + u
            xlo, xhi = 1 + 2 * pt, 2 + 2 * pt
            STbc[(xlo - 1) * NU + u, idx * 128 + np.arange(0, 64)] = 1.0
            STbc[(xhi - 1) * NU + u, idx * 128 + np.arange(64, 128)] = 1.0
    return Gt, ident, I64, Spair, STbc


def _perm_w2(f_W2, f_b2):
    """Split W2 into the t-channel block (x=0) and 4 action pair blocks.

    W2x0 [128, 64]: cols l -> f_W2[:, l*9].
    W2pr [128, 512]: block pt covers x_lo=1+2pt (cols 0..63 of the block)
    and x_hi=2+2pt (cols 64..127).
    """
    W2x0 = np.ascontiguousarray(f_W2[:, 0::XD])            # [128, 64]
    W2pr = np.empty((HID, 4 * HID), np.float32)
    b2T = np.zeros((128, 4), np.float32)
    for pt in range(4):
        for j in range(128):
            x = (1 + 2 * pt) if j < 64 else (2 + 2 * pt)
            l = j % 64
            W2pr[:, pt * 128 + j] = f_W2[:, l * XD + x]
            b2T[j, pt] = f_b2[l * XD + x]
    b2x0 = f_b2[0::XD].astype(np.float32)                  # [64]
    return W2x0, W2pr, b2T, b2x0


def _pad_encw(enc_W):
    out = np.zeros((65, LD), np.float32)
    out[0:40] = enc_W[0:40]
    out[64] = enc_W[40]
    return out


# RK4 with 2 substeps of hstep=0.5: next-stage input z + alpha*k, and
# znew = z + sum beta_s k_s.
ALPHAS = [0.25, 0.25, 0.5]
BETAS = [1.0 / 12.0, 1.0 / 6.0, 1.0 / 6.0, 1.0 / 12.0]


def _build_program(has_b2):
    import concourse.bacc as bacc
    import concourse.bass as bass
    import concourse.mybir as mybir
    import concourse.tile as tile
    from contextlib import ExitStack

    dt = mybir.dt
    F32 = dt.float32
    F32R = dt.float32r
    BF16 = dt.bfloat16
    AF = mybir.ActivationFunctionType
    ALU = mybir.AluOpType

    nc = bacc.Bacc("TRN2", target_bir_lowering=False, debug=False,
                   num_devices=NCORES)

    # ---- DRAM tensors ----
    s0_d = nc.dram_tensor("s0", [BC, SD], F32, kind="ExternalInput").ap()
    a_d = nc.dram_tensor("a_in", [BC, L, AD], F32, kind="ExternalInput").ap()
    t_d = nc.dram_tensor("t_in", [BC, L], F32, kind="ExternalInput").ap()
    W1_d = nc.dram_tensor("W1", [LD, HID], F32R, kind="ExternalInput").ap()
    W2pr_d = nc.dram_tensor("W2pr", [HID, 4 * HID], F32R, kind="ExternalInput").ap()
    b2T_d = nc.dram_tensor("b2T", [128, 4], F32, kind="ExternalInput").ap()
    # W1P/W10 alpha-scaled variants, Spair/W2x0 beta-scaled variants
    W1Pq_d = nc.dram_tensor("W1Pq", [HID, HID], F32R, kind="ExternalInput").ap()
    W1Ph_d = nc.dram_tensor("W1Ph", [HID, HID], F32R, kind="ExternalInput").ap()
    W10q_d = nc.dram_tensor("W10q", [HID, HID], F32R, kind="ExternalInput").ap()
    W10h_d = nc.dram_tensor("W10h", [HID, HID], F32R, kind="ExternalInput").ap()
    Spb1_d = nc.dram_tensor("Spb1", [HID, LD], F32R, kind="ExternalInput").ap()
    Spb2_d = nc.dram_tensor("Spb2", [HID, LD], F32R, kind="ExternalInput").ap()
    W2x0b1_d = nc.dram_tensor("W2x0b1", [HID, LD], F32R, kind="ExternalInput").ap()
    W2x0b2_d = nc.dram_tensor("W2x0b2", [HID, LD], F32R, kind="ExternalInput").ap()
    I64_d = nc.dram_tensor("I64", [LD, LD], F32R, kind="ExternalInput").ap()
    STbc_d = nc.dram_tensor("STbc", [40, 4 * NU * 128], BF16,
                            kind="ExternalInput").ap()
    # relu biases: b1 (+ alpha*W1^T b2x0 variants) ; b2x0 znew row
    b1s_d = nc.dram_tensor("b1s", [HID, 3], F32, kind="ExternalInput").ap()
    b2x0h_d = nc.dram_tensor("b2x0h", [1, LD], F32R, kind="ExternalInput").ap()
    onesr_d = nc.dram_tensor("onesr", [1, BC], F32R, kind="ExternalInput").ap()
    encW_d = nc.dram_tensor("encW", [65, LD], F32R, kind="ExternalInput").ap()
    encb_d = nc.dram_tensor("encb", [LD, 1], F32, kind="ExternalInput").ap()
    decW_d = nc.dram_tensor("decW", [LD, SD], F32R, kind="ExternalInput").ap()
    decb_d = nc.dram_tensor("decb", [SD, 1], F32, kind="ExternalInput").ap()
    Gt_d = nc.dram_tensor("Gt", [NU, L, NI], F32, kind="ExternalInput").ap()
    id_d = nc.dram_tensor("ident", [L, L], F32, kind="ExternalInput").ap()
    zpad_d = nc.dram_tensor("zpad", [24, BC], F32R, kind="ExternalInput").ap()

    outT_d = nc.dram_tensor("outT", [L, SD, BC], F32, kind="ExternalOutput").ap()
    # compact dX scratch: row (ch-1)*NU + u, per interval, per batch
    dxall_d = nc.dram_tensor("dxall", [40, NI, BC], dt.bfloat16).ap()

    mmr = nc.tensor.matmul

    import os as _os_ts
    _TRACE_SIM = _os_ts.environ.get("K_TRACE_SIM", "0") == "1"
    with tile.TileContext(nc, trace_sim=_TRACE_SIM) as tc, ExitStack() as st:
        # ---------- persistent pools ----------
        wp = st.enter_context(tc.tile_pool(name="weights", bufs=1))

        def wtile(name, dram, shape, dtp):
            t = wp.tile(shape, dtp, tag=name)
            nc.sync.dma_start(t[:], dram)
            return t

        W1_s = wtile("W1", W1_d, [LD, HID], F32R)
        W2pr_s = wtile("W2pr", W2pr_d, [HID, 4 * HID], F32R)
        b2T_s = wtile("b2T", b2T_d, [128, 4], F32)
        W1Pq_s = wtile("W1Pq", W1Pq_d, [HID, HID], F32R)
        W1Ph_s = wtile("W1Ph", W1Ph_d, [HID, HID], F32R)
        W10q_s = wtile("W10q", W10q_d, [HID, HID], F32R)
        W10h_s = wtile("W10h", W10h_d, [HID, HID], F32R)
        Spb1_s = wtile("Spb1", Spb1_d, [HID, LD], F32R)
        Spb2_s = wtile("Spb2", Spb2_d, [HID, LD], F32R)
        W2x0b1_s = wtile("W2x0b1", W2x0b1_d, [HID, LD], F32R)
        W2x0b2_s = wtile("W2x0b2", W2x0b2_d, [HID, LD], F32R)
        I64_s = wtile("I64", I64_d, [LD, LD], F32R)
        STbc_s = wtile("STbc", STbc_d, [40, 4 * NU * 128], BF16)
        b1s_s = wtile("b1s", b1s_d, [HID, 3], F32)
        b2x0h_s = wtile("b2x0h", b2x0h_d, [1, LD], F32R)
        onesr_s = wtile("onesr", onesr_d, [1, BC], F32R)
        encW_s = wtile("encW", encW_d, [65, LD], F32R)
        encb_s = wtile("encb", encb_d, [LD, 1], F32)
        decW_s = wtile("decW", decW_d, [LD, SD], F32R)
        decb_s = wtile("decb", decb_d, [SD, 1], F32)
        id_s = wtile("ident", id_d, [L, L], F32)
        Gt_s = wp.tile([L, NU * NI], F32, tag="Gt")
        for u in range(NU):
            nc.sync.dma_start(Gt_s[:, u * NI:(u + 1) * NI], Gt_d[u])

        zp = st.enter_context(tc.tile_pool(name="zstate", bufs=2))
        zst = zp.tile([LD, BC], F32R, tag="z")   # latent, batch-wide T-layout
        # per-interval compact dX slices [40, BC] (bf16), prefetched
        dxcp = st.enter_context(tc.tile_pool(name="dxc_sb", bufs=3))
        # broadcast tiles bt(pt,u) [128, BC] bf16 for current+next interval
        btsp = st.enter_context(tc.tile_pool(name="bt_sb", bufs=44))

        # ---------- phases 1-3 ----------
        with tc.tile_pool(name="ph_sb", bufs=4) as php, \
             tc.tile_pool(name="ph_ps", bufs=4, space="PSUM") as ppp:
            xT = php.tile([L, XD * BC], F32, tag="xT")
            in0T = php.tile([65, BC], F32R, tag="in0T")
            nc.sync.dma_start(in0T[40:64, :], zpad_d)
            for cb in range(4):
                csl = slice(cb * 128, (cb + 1) * 128)
                tb = php.tile([128, L], F32, tag="tb")
                nc.sync.dma_start(tb[:], t_d[csl, :])
                ab = php.tile([128, L * AD], F32, tag="ab")
                nc.sync.dma_start(ab[:], a_d[csl].rearrange("b l c -> b (l c)"))
                sb = php.tile([128, SD], F32, tag="sb")
                nc.sync.dma_start(sb[:], s0_d[csl, :])

                a3 = ab[:].rearrange("b (l c) -> b l c", c=AD)
                for ch in range(AD):
                    pa = ppp.tile([L, 128], F32, tag="tp")
                    nc.tensor.transpose(pa[:], a3[:, :, ch], id_s[:])
                    o = (1 + ch) * BC + cb * 128
                    nc.scalar.copy(xT[:, o:o + 128], pa[:])
                ps = ppp.tile([SD, 128], F32, tag="tp")
                nc.tensor.transpose(ps[:], sb[:], id_s[:])
                nc.scalar.copy(in0T[0:SD, cb * 128: cb * 128 + 128], ps[:])
                pa0 = ppp.tile([AD, 128], F32, tag="tp")
                nc.tensor.transpose(pa0[:], a3[:, 0, :], id_s[:])
                nc.scalar.copy(in0T[SD:SD + AD, cb * 128: cb * 128 + 128], pa0[:])
                pt0 = ppp.tile([1, 128], F32, tag="tp")
                nc.tensor.transpose(pt0[:], tb[:, 0:1], id_s[:])
                nc.scalar.copy(in0T[64:65, cb * 128: cb * 128 + 128], pt0[:])

            # phase 2: dX for action channels (full fp32 matmuls), staged
            # per channel as [NI, NU*BC] bf16 and stored row-wise to the
            # compact DRAM scratch dxall [40, NI, BC].
            for ch in range(1, XD):
                gsbc = php.tile([NI, NU * BC], BF16, tag="gsbc")
                for u in range(NU):
                    pg = ppp.tile([NI, BC], F32, tag="g")
                    mmr(pg[:], Gt_s[:, u * NI:(u + 1) * NI],
                        xT[:, ch * BC:(ch + 1) * BC], start=True, stop=True)
                    nc.scalar.copy(gsbc[:, u * BC:(u + 1) * BC], pg[:])
                for u in range(NU):
                    nc.sync.dma_start(dxall_d[(ch - 1) * NU + u],
                                      gsbc[:, u * BC:(u + 1) * BC])

            # phase 3: encoder z0
            pz = ppp.tile([LD, BC], F32, tag="g")
            mmr(pz[:], encW_s[:], in0T[:], start=True, stop=True)
            nc.scalar.activation(zst[:], pz[:], AF.Identity, bias=encb_s[:])
            # decode l=0
            po = ppp.tile([SD, BC], F32, tag="g")
            mmr(po[:], decW_s[:], zst[:], start=True, stop=True)
            oT0 = php.tile([SD, BC], F32, tag="oT")
            nc.scalar.activation(oT0[:], po[:], AF.Identity, bias=decb_s[:])
            nc.scalar.dma_start(outT_d[0], oT0[:])

        tc.strict_bb_all_engine_barrier()

        # ---------- scan pools ----------
        # PSUM budget (8 banks): ph 2 (h accum + decode), pf 3 (pair-f
        # tiles), pbt 2 (broadcast staging), pzn 1 (znew accum).
        ph = st.enter_context(tc.tile_pool(name="ps_h", bufs=2, space="PSUM"))
        pf = st.enter_context(tc.tile_pool(name="ps_f", bufs=3, space="PSUM"))
        pbt = st.enter_context(tc.tile_pool(name="ps_bt", bufs=2, space="PSUM"))
        pzn = st.enter_context(tc.tile_pool(name="ps_zn", bufs=1, space="PSUM"))
        hp = st.enter_context(tc.tile_pool(name="h_sb", bufs=3))
        pp = st.enter_context(tc.tile_pool(name="p_sb", bufs=6))
        fcp = st.enter_context(tc.tile_pool(name="fc_sb", bufs=3))
        otp = st.enter_context(tc.tile_pool(name="o_sb", bufs=2))

        STT = nc.vector.scalar_tensor_tensor

        def dxc_load(i):
            t = dxcp.tile([40, BC], BF16, tag="dxc")
            base = dxall_d
            src = bass.AP(base.tensor, base.offset + i * BC,
                          [[NI * BC, 40], [1, BC]])
            nc.sync.dma_start(t[:], src)
            return t

        # broadcast job: PE select-matmul from dxc, then copy to bf16 SBUF.
        # cp_sel rotates the copy between ACT and DVE.
        def bt_make(dxc, pt, u, cp_sel):
            idx = pt * NU + u
            bp = pbt.tile([128, BC], F32, tag="btp")
            mmr(bp[:], STbc_s[:, idx * 128:(idx + 1) * 128], dxc[:],
                start=True, stop=True)
            bt = btsp.tile([128, BC], BF16, tag="bt")
            if cp_sel == 0:
                nc.scalar.copy(bt[:], bp[:])
            else:
                nc.vector.tensor_scalar_add(bt[:], bp[:], 0.0)
            return bt

        # job list for one interval's 20 broadcasts, ordered u-major so the
        # earliest-needed tiles are produced first
        JOBS = [(pt, u) for u in range(NU) for pt in range(4)]
        # distribution of next-interval jobs across the 8 stages
        JCUT = [0, 2, 5, 7, 10, 12, 15, 17, 20]

        # prologue: dxc for intervals 0..1, broadcasts for interval 0
        dxc_cur = dxc_load(0)
        dxc_nxt = dxc_load(1)
        bts_cur = {}
        for jn, (pt, u) in enumerate(JOBS):
            bts_cur[(pt, u)] = bt_make(dxc_cur, pt, u, jn % 2)

        zcur = zst
        for i in range(NI):
            if i + 2 < NI:
                dxc_fut = dxc_load(i + 2)
            else:
                dxc_fut = None
            bts_nxt = {}

            def emit_jobs(sidx):
                if i + 1 >= NI:
                    return
                for jn in range(JCUT[sidx], JCUT[sidx + 1]):
                    pt, u = JOBS[jn]
                    bts_nxt[(pt, u)] = bt_make(dxc_nxt, pt, u, jn % 2)

            for sub in range(2):
                uix = [0, 1, 1, 2] if sub == 0 else [2, 3, 3, 4]
                # znew accumulation group, seeded with the substep base z
                # (plus the b2 t-channel bias row, zero when b2 == 0).
                znps = pzn.tile([LD, BC], F32, tag="zn")
                mmr(znps[:], I64_s[:], zcur[:], start=True, stop=False,
                    skip_group_check=True)
                if has_b2:
                    mmr(znps[:], b2x0h_s[:], onesr_s[:], start=False,
                        stop=False, skip_group_check=True)
                hps = ph.tile([HID, BC], F32, tag="h")
                mmr(hps[:], W1_s[:], zcur[:], start=True, stop=True,
                    skip_group_check=True)
                for s in range(4):
                    u = uix[s]
                    last = s == 3
                    # bias: b1 (+ alpha*W1^T b2x0 for stages fed by k-accum)
                    bcol = 0 if s == 0 else (2 if s == 3 else 1)
                    hsb = hp.tile([HID, BC], F32R, tag="h")
                    nc.scalar.activation(hsb[:], hps[:], AF.Relu,
                                         bias=b1s_s[:, bcol:bcol + 1])
                    Spb = Spb2_s if s in (1, 2) else Spb1_s
                    W2x0b = W2x0b2_s if s in (1, 2) else W2x0b1_s
                    # all four pair matmuls (separate PSUM tiles)
                    fts = []
                    for pt in (0, 1, 2, 3):
                        fps = pf.tile([HID, BC], F32, tag="f")
                        mmr(fps[:], W2pr_s[:, pt * 128:(pt + 1) * 128],
                            hsb[:], start=True, stop=True)
                        fts.append(fps)
                    # early hpn/znps seeds (depend only on zcur/hsb)
                    if not last:
                        hpn = ph.tile([HID, BC], F32, tag="h")
                        W1P = W1Ph_s if s == 2 else W1Pq_s
                        W10 = W10h_s if s == 2 else W10q_s
                        mmr(hpn[:], W1_s[:], zcur[:], start=True, stop=False,
                            skip_group_check=True)
                        mmr(hpn[:], W10[:], hsb[:], start=False, stop=False,
                            skip_group_check=True)
                    mmr(znps[:], W2x0b[:], hsb[:], start=False, stop=False,
                        skip_group_check=True)
                    # next-interval broadcast matmuls fill the PE gap while
                    # the elementwise pair products run
                    emit_jobs(sub * 4 + s)
                    # pt0/pt1: stage f to SBUF (ACT), GPSIMD muls;
                    # pt2/pt3: DVE STT straight off PSUM.
                    fsb0 = fcp.tile([HID, BC], F32R, tag="fc")
                    if has_b2:
                        nc.scalar.activation(fsb0[:], fts[0][:], AF.Identity,
                                             bias=b2T_s[:, 0:1])
                    else:
                        nc.scalar.copy(fsb0[:], fts[0][:])
                    fsb1 = fcp.tile([HID, BC], F32R, tag="fc")
                    nc.scalar.activation(fsb1[:], fts[1][:], AF.Identity,
                                         bias=b2T_s[:, 1:2])
                    psb0 = pp.tile([HID, BC], F32R, tag="p")
                    nc.gpsimd.tensor_tensor(psb0[:], fsb0[:],
                                            bts_cur[(0, u)][:], op=ALU.mult)
                    psb1 = pp.tile([HID, BC], F32R, tag="p")
                    nc.gpsimd.tensor_tensor(psb1[:], fsb1[:],
                                            bts_cur[(1, u)][:], op=ALU.mult)
                    psb2 = pp.tile([HID, BC], F32R, tag="p")
                    STT(psb2[:], fts[2][:], b2T_s[:, 2:3],
                        bts_cur[(2, u)][:], op0=ALU.add, op1=ALU.mult)
                    psb3 = pp.tile([HID, BC], F32R, tag="p")
                    STT(psb3[:], fts[3][:], b2T_s[:, 3:4],
                        bts_cur[(3, u)][:], op0=ALU.add, op1=ALU.mult)
                    psbs = [psb0, psb1, psb2, psb3]
                    for pt in range(4):
                        psb = psbs[pt][:]
                        if not last:
                            mmr(hpn[:], W1P[:], psb, start=False,
                                stop=(pt == 3), skip_group_check=True)
                        mmr(znps[:], Spb[:], psb, start=False,
                            stop=(last and pt == 3), skip_group_check=True)
                    if not last:
                        hps = hpn
                znsb = zp.tile([LD, BC], F32R, tag="z")
                nc.scalar.copy(znsb[:], znps[:])
                zcur = znsb

            # decode z_{i+1} (borrows an h PSUM slot; free at interval end)
            pdo = ph.tile([SD, BC], F32, tag="h")
            mmr(pdo[:], decW_s[:], zcur[:], start=True, stop=True)
            oT = otp.tile([SD, BC], F32, tag="oT")
            nc.scalar.activation(oT[:], pdo[:], AF.Identity, bias=decb_s[:])
            nc.scalar.dma_start(outT_d[i + 1], oT[:])

            bts_cur = bts_nxt
            dxc_cur = dxc_nxt
            dxc_nxt = dxc_fut

    nc.compile()
    return nc


def _get_program(has_b2=False):
    key = ("nc", bool(has_b2))
    if key not in _CACHE:
        _CACHE[key] = _build_program(has_b2)
    return _CACHE[key]


def build_in_maps(s, a, t, enc_W, enc_b, f_W1, f_b1, f_W2, f_b2, dec_W, dec_b):
    import ml_dtypes

    s = np.ascontiguousarray(np.asarray(s, np.float32))
    a = np.ascontiguousarray(np.asarray(a, np.float32))
    t = np.ascontiguousarray(np.asarray(t, np.float32))
    Gt, ident, I64, Spair, STbc = _host_consts()
    f_b2 = np.asarray(f_b2, np.float32)
    f_b1 = np.asarray(f_b1, np.float32)
    W1 = np.ascontiguousarray(np.asarray(f_W1, np.float32))
    has_b2 = bool(np.any(f_b2 != 0.0))
    W2x0, W2pr, b2T, b2x0 = _perm_w2(np.asarray(f_W2, np.float32), f_b2)
    W1P = Spair @ W1                     # [128, 128]
    W10 = W2x0 @ W1                      # [128, 128]
    w1tb = W1.T @ b2x0                   # [128] relu-bias correction
    b1s = np.stack([f_b1, f_b1 + 0.25 * w1tb, f_b1 + 0.5 * w1tb], axis=1)
    const_map = dict(
        W1=W1,
        W2pr=W2pr, b2T=b2T,
        W1Pq=np.ascontiguousarray(0.25 * W1P),
        W1Ph=np.ascontiguousarray(0.5 * W1P),
        W10q=np.ascontiguousarray(0.25 * W10),
        W10h=np.ascontiguousarray(0.5 * W10),
        Spb1=np.ascontiguousarray(Spair / 12.0),
        Spb2=np.ascontiguousarray(Spair / 6.0),
        W2x0b1=np.ascontiguousarray(W2x0 / 12.0),
        W2x0b2=np.ascontiguousarray(W2x0 / 6.0),
        I64=I64,
        STbc=STbc.astype(ml_dtypes.bfloat16),
        b1s=np.ascontiguousarray(b1s),
        b2x0h=np.ascontiguousarray((0.5 * b2x0).reshape(1, LD)),
        onesr=np.ones((1, BC), np.float32),
        encW=_pad_encw(np.asarray(enc_W, np.float32)),
        encb=np.asarray(enc_b, np.float32).reshape(LD, 1).copy(),
        decW=np.ascontiguousarray(np.asarray(dec_W, np.float32)),
        decb=np.asarray(dec_b, np.float32).reshape(SD, 1).copy(),
        Gt=Gt, ident=ident,
        zpad=np.zeros((24, BC), np.float32),
    )
    in_maps = []
    for c in range(NCORES):
        rs = slice(c * BC, (c + 1) * BC)
        m = dict(const_map)
        m["s0"] = np.ascontiguousarray(s[rs, 0, :])
        m["a_in"] = np.ascontiguousarray(a[rs])
        m["t_in"] = np.ascontiguousarray(t[rs])
        in_maps.append(m)
    return in_maps, has_b2


def kernel(s, a, t, enc_W, enc_b, f_W1, f_b1, f_W2, f_b2, dec_W, dec_b):
    global LAST_RESULTS
    from concourse.bass_utils import run_bass_kernel_spmd

    in_maps, has_b2 = build_in_maps(s, a, t, enc_W, enc_b, f_W1, f_b1, f_W2,
                                    f_b2, dec_W, dec_b)
    nc = _get_program(has_b2)
    res = run_bass_kernel_spmd(nc, in_maps, core_ids=list(range(NCORES)))
    LAST_RESULTS = res

    out = np.empty((B, L, SD), np.float32)
    for c in range(NCORES):
        oT = res.results[c]["outT"]          # [L, SD, BC]
        out[c * BC:(c + 1) * BC] = oT.transpose(2, 0, 1)
    return out
